# revision 6
# baseline (speedup 1.0000x reference)
"""Trainium2 Bass kernel for BinaryMLP.

reference:
    h = relu(x @ sign(W1).T + b1)   # [B, 128], x: [B, 196]
    h = relu(h @ sign(W2).T + b2)   # [B, 128]
    h = relu(h @ sign(W3).T + b3)   # [B, 128]
    y = h @ W4.T + b4               # [B, 10]

Strategy (pure data parallel over 8 cores, 65536 rows each):
  - Host packs the x shard into three bf16 DRAM tensors so that every big
    DMA spans all 128 SBUF partitions (a [68, N] transfer concentrates on
    4 of 16 SDMA engines and was the baseline's bandwidth ceiling):
      xA [128, B]    dims 0..127, batch-major columns
      xB [128, B/2]  dims 128..191, tile-pair packed (even tile on
                     partitions 0..63, odd tile on 64..127)
      xC [16, B/32]  dims 192..195 of tiles packed 4-at-a-time; resident
                     in SBUF for the whole kernel (2 MB total on chip)
  - L1 contraction split K=196 = 128 + 64 + 4:
      pass A: full-array K=128 matmul per tile
      pass B: two tiles' K=64 remainders run CONCURRENTLY via row tiling
              (tile_position (0,0) and (64,0))
      pass C: four tiles' K=4 leftovers run concurrently at row groups
              (0,0),(32,0),(64,0),(96,0)
    -> 7 matmul slots per 4 tiles instead of 8.
  - PSUM: one rotating pool of [128,1024] fp32 (2-bank) tiles shared by
    L1/L2/L3 (bufs=3 = 6 banks) + 1 bank for the packed head. Every
    relu+bias evacuation covers 1024 columns, amortizing the fixed
    ACT/DVE per-op cost (evacuation throughput is the second wall:
    fp32 PSUM reads are capped at 1 elem/lane/cycle on both engines).
  - Evacuations alternate ScalarE / VectorE.
  - Head (M=10): 8 tiles packed per PSUM bank using 4x column tiling
    x 2 zero-masked W4 variants (as in the previous version). Output is
    stored bf16 in the strip layout yTS[128, .]; host unscrambles and
    adds b4.
"""

import numpy as np
import ml_dtypes

import concourse.bass as bass
from concourse.bass import _add_dep_helper
import concourse.mybir as mybir
import concourse.tile as tile
from concourse import bacc
from concourse.bass_utils import run_bass_kernel_spmd

BF16 = ml_dtypes.bfloat16

B_FULL, D_IN, H, D_OUT = 524288, 196, 128, 10
N_CORES = 8
TB = 512          # batch tile = matmul free dim (one PSUM bank of fp32)
SG = 4            # tiles per supergroup (pass-C packing unit)
PACK = 8          # tiles per head pack / store group
K1A = 128         # L1 pass A contraction
K1B = 64          # L1 pass B contraction (row-tiled pairs)
K1C = 4           # L1 pass C contraction (row-tiled quads)


def build_nc(b_core: int, n_cores: int = N_CORES, noload_opt: bool = True):
    """Build the per-core Bass program (SPMD: same program on all cores)."""
    dt = mybir.dt
    nc = bacc.Bacc(
        "TRN2", target_bir_lowering=False, debug=False, num_devices=n_cores
    )

    n_tiles = b_core // TB
    assert b_core % (PACK * TB) == 0
    n_packs = n_tiles // PACK
    n_sg = n_tiles // SG

    xA = nc.dram_tensor("xA", [128, b_core], dt.bfloat16, kind="ExternalInput").ap()
    xB = nc.dram_tensor(
        "xB", [128, b_core // 2], dt.bfloat16, kind="ExternalInput"
    ).ap()
    xC = nc.dram_tensor(
        "xC", [16, b_core // 4], dt.bfloat16, kind="ExternalInput"
    ).ap()
    w1a = nc.dram_tensor("w1a", [K1A, H], dt.bfloat16, kind="ExternalInput").ap()
    w1b = nc.dram_tensor("w1b", [128, H], dt.bfloat16, kind="ExternalInput").ap()
    w1c = nc.dram_tensor("w1c", [128, H], dt.bfloat16, kind="ExternalInput").ap()
    w2t = nc.dram_tensor("w2t", [H, H], dt.bfloat16, kind="ExternalInput").ap()
    w3t = nc.dram_tensor("w3t", [H, H], dt.bfloat16, kind="ExternalInput").ap()
    w4a = nc.dram_tensor("w4a", [H, 32], dt.bfloat16, kind="ExternalInput").ap()
    w4b = nc.dram_tensor("w4b", [H, 32], dt.bfloat16, kind="ExternalInput").ap()
    b1d = nc.dram_tensor("b1", [H, 1], dt.float32, kind="ExternalInput").ap()
    b2d = nc.dram_tensor("b2", [H, 1], dt.float32, kind="ExternalInput").ap()
    b3d = nc.dram_tensor("b3", [H, 1], dt.float32, kind="ExternalInput").ap()
    # strip-layout output: row 32u+10j+p, cols pk*TB+c  <->  y[(pk*8+4j+u)*TB+c, p]
    yTS = nc.dram_tensor(
        "yTS", [H, n_packs * TB], dt.bfloat16, kind="ExternalOutput"
    ).ap()

    relu = mybir.ActivationFunctionType.Relu

    # xA chunk schedule (columns per load); first chunks smaller for a
    # faster pipeline start.
    xa_chunks = [2048, 2048] + [4096] * ((b_core - 4096) // 4096)
    assert sum(xa_chunks) == b_core
    xb_chunks = [2048, 2048] + [4096] * ((b_core // 2 - 4096) // 4096)
    assert sum(xb_chunks) == b_core // 2

    with tile.TileContext(nc) as tc:
        with (
            tc.tile_pool(name="wpool", bufs=1) as wpool,
            tc.tile_pool(name="xa", bufs=3) as xa_pool,
            tc.tile_pool(name="xb", bufs=3) as xb_pool,
            tc.tile_pool(name="h1p", bufs=6) as h1_pool,
            tc.tile_pool(name="h2p", bufs=6) as h2_pool,
            tc.tile_pool(name="h3p", bufs=6) as h3_pool,
            tc.tile_pool(name="yo", bufs=3) as y_pool,
            tc.tile_pool(name="ps", bufs=3, space="PSUM") as ps_pool,
            tc.tile_pool(name="ps4", bufs=1, space="PSUM") as ps4_pool,
        ):
            # --- resident loads: weights, biases, xC ---
            w1a_sb = wpool.tile([K1A, H], dt.bfloat16)
            nc.gpsimd.dma_start(w1a_sb[:], w1a[:, :])
            w1b_sb = wpool.tile([128, H], dt.bfloat16)  # rows 0-63 & 64-127 = W1b
            nc.gpsimd.dma_start(w1b_sb[:], w1b[:, :])
            w1c_sb = wpool.tile([128, H], dt.bfloat16)  # rows 32u..32u+3 = W1c
            nc.gpsimd.dma_start(w1c_sb[:], w1c[:, :])
            w2_sb = wpool.tile([H, H], dt.bfloat16)
            nc.gpsimd.dma_start(w2_sb[:], w2t[:, :])
            w3_sb = wpool.tile([H, H], dt.bfloat16)
            nc.gpsimd.dma_start(w3_sb[:], w3t[:, :])
            w4_sb = [
                wpool.tile([H, 32], dt.bfloat16, tag=f"w4_{j}", name=f"w4_{j}")
                for j in range(2)
            ]
            nc.gpsimd.dma_start(w4_sb[0][:], w4a[:, :])
            nc.gpsimd.dma_start(w4_sb[1][:], w4b[:, :])
            b_sb = []
            for j, bd in enumerate((b1d, b2d, b3d)):
                b = wpool.tile([H, 1], dt.float32, tag=f"b_{j}", name=f"b_{j}")
                nc.gpsimd.dma_start(b[:], bd[:, :])
                b_sb.append(b)
            # xC resident: [128, b_core/32], partitions 32u+r = dim 192+r of
            # tile 4G+u (cols G*TB+c). Loaded once as 4 [4, .] transfers.
            xc_sb = wpool.tile([128, b_core // 4], dt.bfloat16)
            for u in range(4):
                nc.gpsimd.dma_start(
                    xc_sb[32 * u : 32 * u + 4, :], xC[4 * u : 4 * u + 4, :]
                )

            evac_ctr = [0]

            def relu_evac(h_out, psum_in, bias_sb):
                use_act = evac_ctr[0] % 2 == 0
                evac_ctr[0] += 1
                if use_act:
                    return nc.scalar.activation(
                        h_out[:], psum_in, relu, bias=bias_sb[:]
                    )
                else:
                    return nc.vector.tensor_scalar(
                        h_out[:],
                        psum_in,
                        bias_sb[:],
                        0.0,
                        mybir.AluOpType.add,
                        mybir.AluOpType.max,
                    )

            noload = []  # matmuls that reuse already-loaded weights

            # --- load bookkeeping ---
            xa_t: dict = {}   # tile index -> (sbuf tile, col offset within tile)
            xb_t: dict = {}   # pair index -> (sbuf tile, col offset)

            xa_pos = [0, 0]   # [next chunk idx, col base]
            xb_pos = [0, 0]

            def emit_xa_load():
                if xa_pos[0] >= len(xa_chunks):
                    return
                w = xa_chunks[xa_pos[0]]
                c0 = xa_pos[1]
                xa = xa_pool.tile(
                    [128, w], dt.bfloat16, tag="xa", name=f"xa_{xa_pos[0]}",
                    padded_shape=[128, 4096],
                )
                nc.sync.dma_start(xa[:], xA[:, c0 : c0 + w])
                for t in range(c0 // TB, (c0 + w) // TB):
                    xa_t[t] = (xa, t * TB - c0)
                xa_pos[0] += 1
                xa_pos[1] += w

            def emit_xb_load():
                if xb_pos[0] >= len(xb_chunks):
                    return
                w = xb_chunks[xb_pos[0]]
                c0 = xb_pos[1]
                xb = xb_pool.tile(
                    [128, w], dt.bfloat16, tag="xb", name=f"xb_{xb_pos[0]}",
                    padded_shape=[128, 4096],
                )
                nc.sync.dma_start(xb[:], xB[:, c0 : c0 + w])
                for p in range(c0 // TB, (c0 + w) // TB):
                    xb_t[p] = (xb, p * TB - c0)
                xb_pos[0] += 1
                xb_pos[1] += w

            h1_t: dict = {}   # pair -> h1 tile [128, 1024]
            h2_t: dict = {}
            h3_t: dict = {}   # pair -> (h3 tile, evac handle)

            def stage_L1(g):
                # 4 tiles 4g..4g+3 (pairs 2g, 2g+1): A, B, C passes.
                ps = []
                for q in range(2):  # pair within supergroup
                    pr = 2 * g + q
                    p1 = ps_pool.tile(
                        [H, 2 * TB], dt.float32, tag="ps", name=f"p1_{pr}"
                    )
                    for h in range(2):  # tile within pair
                        t = 4 * g + 2 * q + h
                        xa, off = xa_t[t]
                        mm = nc.tensor.matmul(
                            p1[:, h * TB : (h + 1) * TB],
                            w1a_sb[:],
                            xa[:, off : off + TB],
                            start=True,
                            stop=False,
                        )
                        if (q, h) != (0, 0) and noload_opt:
                            mm.ins.ldweights = False
                            noload.append(mm.ins)
                    ps.append(p1)
                # pass B: per pair, two row-tiled K=64 matmuls (concurrent)
                ldwL = nc.tensor.ldweights(w1b_sb[0:64, :], tile_position=(0, 0))
                ldwH = nc.tensor.ldweights(w1b_sb[64:128, :], tile_position=(64, 0))
                for q in range(2):
                    pr = 2 * g + q
                    xb, off = xb_t.pop(pr)
                    p1 = ps[q]
                    for h in range(2):
                        mm = nc.tensor.matmul(
                            p1[:, h * TB : (h + 1) * TB],
                            w1b_sb[64 * h : 64 * h + 64, :],
                            xb[64 * h : 64 * h + 64, off : off + TB],
                            start=False,
                            stop=False,
                            tile_position=(64 * h, 0),
                            skip_group_check=True,
                        )
                        mm.ins.ldweights = False
                        _add_dep_helper(
                            mm.ins, (ldwH if h else ldwL).ins, False, "B mm after ldw"
                        )
                # pass C: 4 row-tiled K=4 matmuls (concurrent)
                ldwC = []
                for u in range(4):
                    ldw = nc.tensor.ldweights(
                        w1c_sb[32 * u : 32 * u + 4, :], tile_position=(32 * u, 0)
                    )
                    ldwC.append(ldw)
                for u in range(4):
                    q, h = divmod(u, 2)
                    mm = nc.tensor.matmul(
                        ps[q][:, h * TB : (h + 1) * TB],
                        w1c_sb[32 * u : 32 * u + 4, :],
                        xc_sb[32 * u : 32 * u + 4, g * TB : (g + 1) * TB],
                        start=False,
                        stop=True,
                        tile_position=(32 * u, 0),
                        skip_group_check=True,
                    )
                    mm.ins.ldweights = False
                    _add_dep_helper(mm.ins, ldwC[u].ins, False, "C mm after ldw")
                # evacuate both pairs at 1024 width
                for q in range(2):
                    pr = 2 * g + q
                    h1 = h1_pool.tile(
                        [H, 2 * TB], dt.bfloat16, tag="h1", name=f"h1_{pr}"
                    )
                    relu_evac(h1, ps[q][:], b_sb[0])
                    h1_t[pr] = h1

            def stage_mid(g, w_sb, src_t, dst_t, bias, layer):
                # L2 or L3 for supergroup g: per pair one [128,1024] psum,
                # two K=128 matmuls, one 1024-wide evac.
                for q in range(2):
                    pr = 2 * g + q
                    hin = src_t.pop(pr)
                    p = ps_pool.tile(
                        [H, 2 * TB], dt.float32, tag="ps", name=f"p{layer}_{pr}"
                    )
                    for h in range(2):
                        mm = nc.tensor.matmul(
                            p[:, h * TB : (h + 1) * TB],
                            w_sb[:],
                            hin[:, h * TB : (h + 1) * TB],
                            start=True,
                            stop=True,
                        )
                        if (q, h) != (0, 0) and noload_opt:
                            mm.ins.ldweights = False
                            noload.append(mm.ins)
                    hout = (h2_pool if layer == 2 else h3_pool).tile(
                        [H, 2 * TB], dt.bfloat16, tag=f"h{layer}",
                        name=f"h{layer}_{pr}",
                    )
                    e = relu_evac(hout, p[:], bias)
                    dst_t[pr] = (hout, e) if layer == 3 else hout

            p4_t: dict = {}

            def stage_head(g):
                # head for supergroup g = variant j of pack pk.
                pk, j = divmod(g, 2)
                if j == 0:
                    p4_t[pk] = ps4_pool.tile(
                        [H, TB], dt.float32, tag="p4", name=f"p4_{pk}"
                    )
                p4 = p4_t[pk]
                hs = [h3_t[2 * g], h3_t[2 * g + 1]]  # two pairs, [128,1024] each
                ldws = []
                for u in range(4):
                    ldw = nc.tensor.ldweights(
                        w4_sb[j][:], tile_position=(0, 32 * u)
                    )
                    for _, e3 in hs:
                        _add_dep_helper(ldw.ins, e3.ins, True, "head ldw after e3")
                    ldws.append(ldw)
                for u in range(4):
                    q, h = divmod(u, 2)
                    mm = nc.tensor.matmul(
                        p4[32 * u : 32 * u + 32, :],
                        w4_sb[j][:],
                        hs[q][0][:, h * TB : (h + 1) * TB],
                        start=(j == 0),
                        stop=(j == 1),
                        tile_position=(0, 32 * u),
                        skip_group_check=True,
                    )
                    mm.ins.ldweights = False
                    _add_dep_helper(mm.ins, ldws[u].ins, False, "head mm after ldw")
                h3_t.pop(2 * g)
                h3_t.pop(2 * g + 1)
                if j == 1:
                    p4 = p4_t.pop(pk)
                    ysb = y_pool.tile(
                        [H, TB], dt.bfloat16, tag="ysb", name=f"ysb_{pk}"
                    )
                    nc.scalar.copy(ysb[:], p4[:])
                    nc.gpsimd.dma_start(yTS[:, pk * TB : (pk + 1) * TB], ysb[:])

            # --- software-pipelined emission at supergroup granularity ---
            # prime the pipeline: 2 xa + 2 xb chunks before any compute
            # (the first two chunks of each are 2048 cols = 1 SG / 2 SGs)
            emit_xa_load()
            emit_xa_load()
            emit_xb_load()
            emit_xb_load()
            for step in range(n_sg + 2):
                # steady state: one xa 4096-chunk (2 SGs of data) every 2
                # steps, one xb 4096-chunk (4 SGs) every 4 steps, always
                # ~2 SGs ahead of consumption with <=3 slots outstanding.
                if step % 2 == 0:
                    emit_xa_load()
                if step % 4 == 2:
                    emit_xb_load()
                if step < n_sg:
                    stage_L1(step)
                if 0 <= step - 1:
                    g = step - 1
                    if g < n_sg:
                        stage_mid(g, w2_sb, h1_t, h2_t, b_sb[1], 2)
                if 0 <= step - 2:
                    g = step - 2
                    if g < n_sg:
                        stage_mid(g, w3_sb, h2_t, h3_t, b_sb[2], 3)
                        stage_head(g)

    nc.compile()
    if noload_opt:
        try:
            _verify_noload_safety(nc, noload)
        except AssertionError:
            # schedule changed in a way that makes weight reuse unsafe;
            # rebuild without the optimization (correctness first)
            return build_nc(b_core, n_cores, noload_opt=False)
    return nc


def _weights_key(inst, idx):
    return str(inst.ins[idx])


def _verify_noload_safety(nc, noload):
    """The schedule is static: verify no other weight load lands between a
    ldweights=False matmul and the instruction that loaded its weights.
    Conservative: ANY intervening load (full, row-partial, or col-partial)
    counts as a clobber."""
    import concourse.mybir as mybir

    noload_ids = {id(i) for i in noload}
    cur = None  # key of the last full/partial weight load
    checked = 0
    insts = []
    for blk in nc.m.functions[0].blocks:
        insts.extend(blk.instructions)
    for inst in insts:
        if inst.engine != mybir.EngineType.PE:
            continue
        kind = type(inst).__name__
        if kind == "InstLdweights":
            tp = getattr(inst, "tile_position", None)
            if not tp or tuple(tp) == (0, 0):
                cur = _weights_key(inst, 0)
            else:
                cur = ("partial", None)
        elif kind == "InstMatmult":
            if id(inst) in noload_ids:
                want = _weights_key(inst, 1)
                assert cur == want, (
                    f"noload matmul {inst.name} expects weights {want}, "
                    f"array has {cur}"
                )
                checked += 1
            elif getattr(inst, "ldweights", None) is False:
                cur = ("partial", None)  # explicit-ldw matmul; its own deps
            else:
                tp = getattr(inst, "tile_position", None)
                if not tp or tuple(tp) == (0, 0):
                    cur = _weights_key(inst, 1)
                else:
                    cur = ("partial", None)
    assert checked == len(noload), (checked, len(noload))


def _prep_core_inputs(x_shard: np.ndarray, weights: dict) -> dict:
    b = x_shard.shape[0]
    xb16 = x_shard.astype(BF16)
    xA = np.ascontiguousarray(xb16[:, 0:128].T)
    v = xb16[:, 128:192].reshape(b // 1024, 2, TB, 64)
    xB = np.ascontiguousarray(v.transpose(1, 3, 0, 2).reshape(128, b // 2))
    w = xb16[:, 192:196].reshape(b // (SG * TB), SG, TB, 4)
    xC = np.ascontiguousarray(w.transpose(1, 3, 0, 2).reshape(16, b // 4))
    return {"xA": xA, "xB": xB, "xC": xC, **weights}


def _prep_weights(W1, b1, W2, b2, W3, b3, W4) -> dict:
    s1 = np.sign(W1).astype(np.float32)
    w1b = np.zeros((128, H), np.float32)
    w1b[0:64] = s1[:, 128:192].T
    w1b[64:128] = s1[:, 128:192].T
    w1c = np.zeros((128, H), np.float32)
    for u in range(4):
        w1c[32 * u : 32 * u + 4] = s1[:, 192:196].T
    w4a = np.zeros((32, H), np.float32)
    w4a[0:D_OUT] = W4
    w4b = np.zeros((32, H), np.float32)
    w4b[D_OUT : 2 * D_OUT] = W4
    return {
        "w1a": np.ascontiguousarray(s1[:, 0:128].T).astype(BF16),
        "w1b": w1b.astype(BF16),
        "w1c": w1c.astype(BF16),
        "w2t": np.ascontiguousarray(np.sign(W2).T).astype(BF16),
        "w3t": np.ascontiguousarray(np.sign(W3).T).astype(BF16),
        "w4a": np.ascontiguousarray(w4a.T).astype(BF16),
        "w4b": np.ascontiguousarray(w4b.T).astype(BF16),
        "b1": b1.reshape(H, 1).astype(np.float32),
        "b2": b2.reshape(H, 1).astype(np.float32),
        "b3": b3.reshape(H, 1).astype(np.float32),
    }


def _unscramble(yTS: np.ndarray, b_core: int) -> np.ndarray:
    """yTS [128, n_packs*TB] strip layout -> y_core [b_core, 10]."""
    n_packs = b_core // (PACK * TB)
    yf = np.asarray(yTS, dtype=np.float32)
    # yTS[32u+10j+p, pk*TB+c] = y[(pk*8+4j+u)*TB + c, p]
    v = yf.reshape(4, 32, n_packs, TB)[:, :20]  # [u, 10j+p, pk, c]
    v = v.reshape(4, 2, 10, n_packs, TB)  # [u, j, p, pk, c]
    y = v.transpose(3, 1, 0, 4, 2).reshape(b_core, D_OUT)
    return y


_NC_CACHE: dict = {}


def run(x, W1, b1, W2, b2, W3, b3, W4, b4, trace=False, trace_kwargs=None):
    """Run the SPMD kernel on 8 cores; returns (y, BassKernelResults)."""
    x = np.asarray(x, dtype=np.float32)
    b_total = x.shape[0]
    assert b_total % N_CORES == 0
    b_core = b_total // N_CORES

    key = b_core
    if key not in _NC_CACHE:
        _NC_CACHE[key] = build_nc(b_core)
    nc = _NC_CACHE[key]

    weights = _prep_weights(
        np.asarray(W1), np.asarray(b1), np.asarray(W2), np.asarray(b2),
        np.asarray(W3), np.asarray(b3), np.asarray(W4),
    )
    in_maps = [
        _prep_core_inputs(x[c * b_core : (c + 1) * b_core], weights)
        for c in range(N_CORES)
    ]
    res = run_bass_kernel_spmd(
        nc,
        in_maps,
        list(range(N_CORES)),
        trace=trace,
        **(trace_kwargs or {}),
    )
    b4f = np.asarray(b4, dtype=np.float32)
    y = np.empty((b_total, D_OUT), dtype=np.float32)
    for c in range(N_CORES):
        y[c * b_core : (c + 1) * b_core] = _unscramble(res.results[c]["yTS"], b_core)
    y += b4f
    return y, res


def kernel(x, W1, b1, W2, b2, W3, b3, W4, b4):
    y, _ = run(x, W1, b1, W2, b2, W3, b3, W4, b4)
    return y


# revision 9
# speedup vs baseline: 1.3711x; 1.3711x over previous
"""Trainium2 Bass kernel for BinaryMLP.

reference:
    h = relu(x @ sign(W1).T + b1)   # [B, 128], x: [B, 196]
    h = relu(h @ sign(W2).T + b2)   # [B, 128]
    h = relu(h @ sign(W3).T + b3)   # [B, 128]
    y = h @ W4.T + b4               # [B, 10] (full-precision head)

Strategy (pure data parallel over 8 cores, 65536 rows each):
  - Host packs the x shard into three bf16 DRAM tensors so every large
    DMA spans all 128 SBUF partitions (a [68, N] transfer lands on only
    4 of 16 SDMA engines and was the original bandwidth ceiling):
      xA [128, B]    x dims 0..127, batch-major columns
      xB [128, B/2]  x dims 128..191, tile-pair packed (even tile of the
                     pair on partitions 0..63, odd tile on 64..127)
      xC [8, B/2]    x dims 192..195, pair packed on partition groups
                     {0..3} (even tile) and {32..35} (odd tile); kept
                     resident in SBUF for the whole kernel
  - L1 contraction split K=196 = 128 + 64 + 4 per tile:
      pass A: full-array K=128 matmul
      pass B: the pair's two K=64 remainders run concurrently via row
              tiling at tile_position (0,0) / (64,0); the stationary
              operand is a single [128,128] image holding W1b twice
      pass C: the pair's two K=4 leftovers run concurrently at
              (0,0) / (32,0) from a [128,128] image holding W1c copies
    The framework emits a correctly-positioned LDWEIGHTS before every
    matmul; the PE's 64-deep reorder window hides them.
  - PSUM: ps1 bufs=3 (L1), ps2/ps3 bufs=2 (L2/L3), 1 head bank.
    All relu+bias evacuations are [128,512], alternating ScalarE and
    VectorE (fp32 PSUM reads run at 1 elem/lane/cycle on both engines,
    which makes the two evacuation engines the throughput wall).
  - Software pipelining: per-pair stages with a 2-pair skew (L2 of pair
    i-2, L1 of pair i, L3/head of pair i-4 per step).
  - Head (M=10): 8 tiles per PSUM bank via 4x column tiling x 2
    zero-masked W4 variants packed in a [128,128] stationary image.
    Output stays in the strip layout yTS[128, .] (rows 32u+10j+p),
    stored bf16; the host unscrambles and adds b4.
"""

import numpy as np
import ml_dtypes

import concourse.bass as bass
from concourse.bass import _add_dep_helper
import concourse.mybir as mybir
import concourse.tile as tile
from concourse import bacc
from concourse.bass_utils import run_bass_kernel_spmd

BF16 = ml_dtypes.bfloat16

B_FULL, D_IN, H, D_OUT = 524288, 196, 128, 10
N_CORES = 8
TB = 512          # batch tile = matmul free dim (one PSUM bank of fp32)
PACK = 8          # tiles per head pack / store group
K1A = 128


def build_nc(b_core: int, n_cores: int = N_CORES):
    """Build the per-core Bass program (SPMD: same program on all cores)."""
    dt = mybir.dt
    nc = bacc.Bacc(
        "TRN2", target_bir_lowering=False, debug=False, num_devices=n_cores
    )

    n_tiles = b_core // TB
    assert b_core % (PACK * TB) == 0
    n_packs = n_tiles // PACK
    n_pairs = n_tiles // 2

    xA = nc.dram_tensor("xA", [128, b_core], dt.bfloat16, kind="ExternalInput").ap()
    xB = nc.dram_tensor(
        "xB", [128, b_core // 2], dt.bfloat16, kind="ExternalInput"
    ).ap()
    xC = nc.dram_tensor(
        "xC", [8, b_core // 2], dt.bfloat16, kind="ExternalInput"
    ).ap()
    w1a = nc.dram_tensor("w1a", [K1A, H], dt.bfloat16, kind="ExternalInput").ap()
    w1b = nc.dram_tensor("w1b", [128, H], dt.bfloat16, kind="ExternalInput").ap()
    w1c = nc.dram_tensor("w1c", [128, H], dt.bfloat16, kind="ExternalInput").ap()
    w2t = nc.dram_tensor("w2t", [H, H], dt.bfloat16, kind="ExternalInput").ap()
    w3t = nc.dram_tensor("w3t", [H, H], dt.bfloat16, kind="ExternalInput").ap()
    w4a = nc.dram_tensor("w4a", [H, 128], dt.bfloat16, kind="ExternalInput").ap()
    w4b = nc.dram_tensor("w4b", [H, 128], dt.bfloat16, kind="ExternalInput").ap()
    b1d = nc.dram_tensor("b1", [H, 1], dt.float32, kind="ExternalInput").ap()
    b2d = nc.dram_tensor("b2", [H, 1], dt.float32, kind="ExternalInput").ap()
    b3d = nc.dram_tensor("b3", [H, 1], dt.float32, kind="ExternalInput").ap()
    # strip-layout output: row 32u+10j+p, cols pk*TB+c  <->  y[(pk*8+4j+u)*TB+c, p]
    yTS = nc.dram_tensor(
        "yTS", [H, n_packs * TB], dt.bfloat16, kind="ExternalOutput"
    ).ap()

    relu = mybir.ActivationFunctionType.Relu

    # chunked loads; first chunks smaller for a faster pipeline start
    xa_chunks = [2048, 2048] + [4096] * ((b_core - 4096) // 4096)
    xb_chunks = [2048, 2048] + [4096] * ((b_core // 2 - 4096) // 4096)

    with tile.TileContext(nc) as tc:
        with (
            tc.tile_pool(name="wpool", bufs=1) as wpool,
            tc.tile_pool(name="xa", bufs=3) as xa_pool,
            tc.tile_pool(name="xb", bufs=3) as xb_pool,
            tc.tile_pool(name="h1p", bufs=9) as h1_pool,
            tc.tile_pool(name="h2p", bufs=9) as h2_pool,
            tc.tile_pool(name="h3p", bufs=14) as h3_pool,
            tc.tile_pool(name="yo", bufs=3) as y_pool,
            tc.tile_pool(name="ps1", bufs=3, space="PSUM") as ps1,
            tc.tile_pool(name="ps2", bufs=2, space="PSUM") as ps2,
            tc.tile_pool(name="ps3", bufs=2, space="PSUM") as ps3,
            tc.tile_pool(name="ps4", bufs=1, space="PSUM") as ps4,
        ):
            # --- resident loads: weights, biases, xC ---
            w1a_sb = wpool.tile([K1A, H], dt.bfloat16)
            nc.gpsimd.dma_start(w1a_sb[:], w1a[:, :])
            w1b_sb = wpool.tile([128, H], dt.bfloat16)
            nc.gpsimd.dma_start(w1b_sb[:], w1b[:, :])
            w1c_sb = wpool.tile([128, H], dt.bfloat16)
            nc.gpsimd.dma_start(w1c_sb[:], w1c[:, :])
            w2_sb = wpool.tile([H, H], dt.bfloat16)
            nc.gpsimd.dma_start(w2_sb[:], w2t[:, :])
            w3_sb = wpool.tile([H, H], dt.bfloat16)
            nc.gpsimd.dma_start(w3_sb[:], w3t[:, :])
            w4_sb = [
                wpool.tile([H, 128], dt.bfloat16, tag=f"w4_{j}", name=f"w4_{j}")
                for j in range(2)
            ]
            nc.gpsimd.dma_start(w4_sb[0][:], w4a[:, :])
            nc.gpsimd.dma_start(w4_sb[1][:], w4b[:, :])
            b_sb = []
            for j, bd in enumerate((b1d, b2d, b3d)):
                b = wpool.tile([H, 1], dt.float32, tag=f"b_{j}", name=f"b_{j}")
                nc.gpsimd.dma_start(b[:], bd[:, :])
                b_sb.append(b)
            # xC resident: partitions {0..3} even tile dims 192..195,
            # {32..35} odd tile; cols = pair*TB + c
            xc_sb = wpool.tile([128, b_core // 2], dt.bfloat16)
            nc.gpsimd.dma_start(xc_sb[0:4, :], xC[0:4, :])
            nc.gpsimd.dma_start(xc_sb[32:36, :], xC[4:8, :])

            evac_ctr = [0]

            def relu_evac(h_out, psum_in, bias_sb):
                use_act = evac_ctr[0] % 2 == 0
                evac_ctr[0] += 1
                if use_act:
                    return nc.scalar.activation(
                        h_out[:], psum_in, relu, bias=bias_sb[:]
                    )
                else:
                    return nc.vector.tensor_scalar(
                        h_out[:],
                        psum_in,
                        bias_sb[:],
                        0.0,
                        mybir.AluOpType.add,
                        mybir.AluOpType.max,
                    )

            # --- load bookkeeping ---
            xa_t: dict = {}   # tile -> (sbuf tile, col offset)
            xb_t: dict = {}   # pair -> (sbuf tile, col offset)
            xa_pos = [0, 0]
            xb_pos = [0, 0]

            def emit_xa_load():
                if xa_pos[0] >= len(xa_chunks):
                    return
                w = xa_chunks[xa_pos[0]]
                c0 = xa_pos[1]
                xa = xa_pool.tile(
                    [128, w], dt.bfloat16, tag="xa", name=f"xa_{xa_pos[0]}",
                    padded_shape=[128, 4096],
                )
                nc.sync.dma_start(xa[:], xA[:, c0 : c0 + w])
                for t in range(c0 // TB, (c0 + w) // TB):
                    xa_t[t] = (xa, t * TB - c0)
                xa_pos[0] += 1
                xa_pos[1] += w

            def emit_xb_load():
                if xb_pos[0] >= len(xb_chunks):
                    return
                w = xb_chunks[xb_pos[0]]
                c0 = xb_pos[1]
                xb = xb_pool.tile(
                    [128, w], dt.bfloat16, tag="xb", name=f"xb_{xb_pos[0]}",
                    padded_shape=[128, 4096],
                )
                nc.sync.dma_start(xb[:], xB[:, c0 : c0 + w])
                for p in range(c0 // TB, (c0 + w) // TB):
                    xb_t[p] = (xb, p * TB - c0)
                xb_pos[0] += 1
                xb_pos[1] += w

            h1_t: dict = {}
            h2_t: dict = {}
            h3_t: dict = {}

            def stage_A(i):
                # L1 for pair i: A (K=128) per tile, then the pair's B
                # (row-tiled K=64 x2) and C (row-tiled K=4 x2) passes.
                xb, boff = xb_t.pop(i)
                ps = []
                for q in range(2):
                    t = 2 * i + q
                    xa, off = xa_t[t]
                    p1 = ps1.tile([H, TB], dt.float32, tag="p1", name=f"p1_{t}")
                    nc.tensor.matmul(
                        p1[:], w1a_sb[:], xa[:, off : off + TB],
                        start=True, stop=False,
                    )
                    ps.append(p1)
                for q in range(2):
                    nc.tensor.matmul(
                        ps[q][:],
                        w1b_sb[64 * q : 64 * q + 64, :],
                        xb[64 * q : 64 * q + 64, boff : boff + TB],
                        start=False, stop=False,
                        tile_position=(64 * q, 0),
                        skip_group_check=True,
                    )
                for q in range(2):
                    nc.tensor.matmul(
                        ps[q][:],
                        w1c_sb[32 * q : 32 * q + 4, :],
                        xc_sb[32 * q : 32 * q + 4, i * TB : (i + 1) * TB],
                        start=False, stop=True,
                        tile_position=(32 * q, 0),
                        skip_group_check=True,
                    )
                for q in range(2):
                    t = 2 * i + q
                    h1 = h1_pool.tile([H, TB], dt.bfloat16, tag="h1", name=f"h1_{t}")
                    relu_evac(h1, ps[q][:], b_sb[0])
                    h1_t[t] = h1

            def stage_B(i):  # L2 for pair i
                for q in range(2):
                    t = 2 * i + q
                    h1 = h1_t.pop(t)
                    p2 = ps2.tile([H, TB], dt.float32, tag="p2", name=f"p2_{t}")
                    nc.tensor.matmul(p2[:], w2_sb[:], h1[:], start=True, stop=True)
                    h2 = h2_pool.tile([H, TB], dt.bfloat16, tag="h2", name=f"h2_{t}")
                    relu_evac(h2, p2[:], b_sb[1])
                    h2_t[t] = h2

            def stage_C(i):  # L3 for pair i
                for q in range(2):
                    t = 2 * i + q
                    h2 = h2_t.pop(t)
                    p3 = ps3.tile([H, TB], dt.float32, tag="p3", name=f"p3_{t}")
                    nc.tensor.matmul(p3[:], w3_sb[:], h2[:], start=True, stop=True)
                    h3 = h3_pool.tile([H, TB], dt.bfloat16, tag="h3", name=f"h3_{t}")
                    e3 = relu_evac(h3, p3[:], b_sb[2])
                    h3_t[t] = (h3, e3)

            p4_t: dict = {}

            def stage_Hj(pk, j):
                # head burst: variant j covers tiles 8pk+4j+u (u=0..3) --
                # pairs 4pk+2j, 4pk+2j+1, both freshly evacuated, so the 4
                # col-tiled matmuls co-issue. Stationary = [128,128] image
                # holding 4 copies of the zero-masked W4 variant.
                if j == 0:
                    p4_t[pk] = ps4.tile([H, TB], dt.float32, tag="p4", name=f"p4_{pk}")
                p4 = p4_t[pk]
                tiles = [8 * pk + 4 * j + u for u in range(4)]
                hs = [h3_t[t] for t in tiles]
                ldw = nc.tensor.ldweights(w4_sb[j][:])
                for _, e3 in hs:
                    _add_dep_helper(ldw.ins, e3.ins, True, "head ldw after e3")
                for u in range(4):
                    mm = nc.tensor.matmul(
                        p4[32 * u : 32 * u + 32, :],
                        w4_sb[j][:, 32 * u : 32 * u + 32],
                        hs[u][0][:],
                        start=(j == 0),
                        stop=(j == 1),
                        tile_position=(0, 32 * u),
                        skip_group_check=True,
                    )
                    mm.ins.ldweights = False
                    _add_dep_helper(mm.ins, ldw.ins, False, "head mm after ldw")
                for t in tiles:
                    h3_t.pop(t)
                if j == 1:
                    p4 = p4_t.pop(pk)
                    ysb = y_pool.tile([H, TB], dt.bfloat16, tag="ysb", name=f"ysb_{pk}")
                    nc.scalar.copy(ysb[:], p4[:])
                    nc.gpsimd.dma_start(yTS[:, pk * TB : (pk + 1) * TB], ysb[:])

            # --- software-pipelined emission (pair granularity, as in the
            # original: L1 of pair i, L2 of pair i-2, L3/head of pair i-4) ---
            emit_xa_load()   # tiles 0..3   (pairs 0-1)
            emit_xa_load()   # tiles 4..7   (pairs 2-3)
            emit_xb_load()   # pairs 0..3
            emit_xb_load()   # pairs 4..7
            for step in range(n_pairs + 4):
                # xa 4096-chunk = 8 tiles = 4 pairs -> one load per 4 steps;
                # xb 4096-chunk = 8 pairs -> one load per 8 steps.
                if step % 4 == 0:
                    emit_xa_load()
                if step % 8 == 4:
                    emit_xb_load()
                ib = step - 2
                if 0 <= ib < n_pairs:
                    stage_B(ib)
                ic = step - 4
                if 0 <= ic < n_pairs:
                    stage_C(ic)
                    if ic % 4 == 1:
                        stage_Hj(ic // 4, 0)
                    elif ic % 4 == 3:
                        stage_Hj(ic // 4, 1)
                if step < n_pairs:
                    stage_A(step)

    nc.compile()
    return nc


def _prep_core_inputs(x_shard: np.ndarray, weights: dict) -> dict:
    b = x_shard.shape[0]
    xb16 = x_shard.astype(BF16)
    xA = np.ascontiguousarray(xb16[:, 0:128].T)
    v = xb16[:, 128:192].reshape(b // 1024, 2, TB, 64)
    xB = np.ascontiguousarray(v.transpose(1, 3, 0, 2).reshape(128, b // 2))
    w = xb16[:, 192:196].reshape(b // 1024, 2, TB, 4)
    xC = np.ascontiguousarray(w.transpose(1, 3, 0, 2).reshape(8, b // 2))
    return {"xA": xA, "xB": xB, "xC": xC, **weights}


def _prep_weights(W1, b1, W2, b2, W3, b3, W4) -> dict:
    s1 = np.sign(W1).astype(np.float32)
    w1b = np.zeros((128, H), np.float32)
    w1b[0:64] = s1[:, 128:192].T
    w1b[64:128] = s1[:, 128:192].T
    w1c = np.zeros((128, H), np.float32)
    w1c[0:4] = s1[:, 192:196].T
    w1c[32:36] = s1[:, 192:196].T
    w4a = np.zeros((32, H), np.float32)
    w4a[0:D_OUT] = W4
    w4b = np.zeros((32, H), np.float32)
    w4b[D_OUT : 2 * D_OUT] = W4
    return {
        "w1a": np.ascontiguousarray(s1[:, 0:128].T).astype(BF16),
        "w1b": w1b.astype(BF16),
        "w1c": w1c.astype(BF16),
        "w2t": np.ascontiguousarray(np.sign(W2).T).astype(BF16),
        "w3t": np.ascontiguousarray(np.sign(W3).T).astype(BF16),
        "w4a": np.ascontiguousarray(np.tile(w4a.T, (1, 4))).astype(BF16),
        "w4b": np.ascontiguousarray(np.tile(w4b.T, (1, 4))).astype(BF16),
        "b1": b1.reshape(H, 1).astype(np.float32),
        "b2": b2.reshape(H, 1).astype(np.float32),
        "b3": b3.reshape(H, 1).astype(np.float32),
    }


def _unscramble(yTS: np.ndarray, b_core: int) -> np.ndarray:
    """yTS [128, n_packs*TB] strip layout -> y_core [b_core, 10]."""
    n_packs = b_core // (PACK * TB)
    yf = np.asarray(yTS, dtype=np.float32)
    # yTS[32u+10j+p, pk*TB+c] = y[(pk*8+4j+u)*TB + c, p]
    v = yf.reshape(4, 32, n_packs, TB)[:, :20]  # [u, 10j+p, pk, c]
    v = v.reshape(4, 2, 10, n_packs, TB)  # [u, j, p, pk, c]
    y = v.transpose(3, 1, 0, 4, 2).reshape(b_core, D_OUT)
    return y


_NC_CACHE: dict = {}


def run(x, W1, b1, W2, b2, W3, b3, W4, b4, trace=False, trace_kwargs=None):
    """Run the SPMD kernel on 8 cores; returns (y, BassKernelResults)."""
    x = np.asarray(x, dtype=np.float32)
    b_total = x.shape[0]
    assert b_total % N_CORES == 0
    b_core = b_total // N_CORES

    key = b_core
    if key not in _NC_CACHE:
        _NC_CACHE[key] = build_nc(b_core)
    nc = _NC_CACHE[key]

    weights = _prep_weights(
        np.asarray(W1), np.asarray(b1), np.asarray(W2), np.asarray(b2),
        np.asarray(W3), np.asarray(b3), np.asarray(W4),
    )
    in_maps = [
        _prep_core_inputs(x[c * b_core : (c + 1) * b_core], weights)
        for c in range(N_CORES)
    ]
    res = run_bass_kernel_spmd(
        nc,
        in_maps,
        list(range(N_CORES)),
        trace=trace,
        **(trace_kwargs or {}),
    )
    b4f = np.asarray(b4, dtype=np.float32)
    y = np.empty((b_total, D_OUT), dtype=np.float32)
    for c in range(N_CORES):
        y[c * b_core : (c + 1) * b_core] = _unscramble(res.results[c]["yTS"], b_core)
    y += b4f
    return y, res


def kernel(x, W1, b1, W2, b2, W3, b3, W4, b4):
    y, _ = run(x, W1, b1, W2, b2, W3, b3, W4, b4)
    return y


# revision 10
# speedup vs baseline: 1.5311x; 1.1167x over previous
"""Trainium2 Bass kernel for BinaryMLP.

reference:
    h = relu(x @ sign(W1).T + b1)   # [B, 128], x: [B, 196]
    h = relu(h @ sign(W2).T + b2)   # [B, 128]
    h = relu(h @ sign(W3).T + b3)   # [B, 128]
    y = h @ W4.T + b4               # [B, 10] (full-precision head)

Strategy (pure data parallel over 8 cores, 65536 rows each):
  - Host packs the x shard into three bf16 DRAM tensors so every large
    DMA spans all 128 SBUF partitions (a [68, N] transfer lands on only
    4 of 16 SDMA engines and was the original bandwidth ceiling):
      xA [128, B]    x dims 0..127, batch-major columns
      xB [128, B/2]  x dims 128..191, tile-pair packed (even tile of the
                     pair on partitions 0..63, odd tile on 64..127)
      xC [8, B/2]    x dims 192..195, pair packed on partition groups
                     {0..3} (even tile) and {32..35} (odd tile); kept
                     resident in SBUF for the whole kernel
  - L1 contraction split K=196 = 128 + 64 + 4 per tile:
      pass A: full-array K=128 matmul
      pass B: the pair's two K=64 remainders run concurrently via row
              tiling at tile_position (0,0) / (64,0); the stationary
              operand is a single [128,128] image holding W1b twice
      pass C: the pair's two K=4 leftovers run concurrently at
              (0,0) / (32,0) from a [128,128] image holding W1c copies
    The framework emits a correctly-positioned LDWEIGHTS before every
    matmul; the PE's 64-deep reorder window hides them.
  - PSUM: ps1 bufs=3 (L1), ps2/ps3 bufs=2 (L2/L3), 1 head bank.
    All relu+bias evacuations are [128,512], alternating ScalarE and
    VectorE (fp32 PSUM reads run at 1 elem/lane/cycle on both engines,
    which makes the two evacuation engines the throughput wall).
  - Software pipelining: per-pair stages with a 2-pair skew (L2 of pair
    i-2, L1 of pair i, L3/head of pair i-4 per step).
  - Head (M=10): 8 tiles per PSUM bank via 4x column tiling x 2
    zero-masked W4 variants packed in a [128,128] stationary image.
    Output stays in the strip layout yTS[128, .] (rows 32u+10j+p),
    stored bf16; the host unscrambles and adds b4.
"""

import numpy as np
import ml_dtypes

import concourse.bass as bass
from concourse.bass import _add_dep_helper
import concourse.mybir as mybir
import concourse.tile as tile
from concourse import bacc
from concourse.bass_utils import run_bass_kernel_spmd

BF16 = ml_dtypes.bfloat16

B_FULL, D_IN, H, D_OUT = 524288, 196, 128, 10
N_CORES = 8
TB = 512          # batch tile = matmul free dim (one PSUM bank of fp32)
PACK = 8          # tiles per head pack / store group
K1A = 128


def build_nc(b_core: int, n_cores: int = N_CORES):
    """Build the per-core Bass program (SPMD: same program on all cores)."""
    dt = mybir.dt
    nc = bacc.Bacc(
        "TRN2", target_bir_lowering=False, debug=False, num_devices=n_cores
    )

    n_tiles = b_core // TB
    assert b_core % (PACK * TB) == 0
    n_packs = n_tiles // PACK
    n_pairs = n_tiles // 2

    xA = nc.dram_tensor("xA", [128, b_core], dt.bfloat16, kind="ExternalInput").ap()
    xB = nc.dram_tensor(
        "xB", [128, b_core // 2], dt.bfloat16, kind="ExternalInput"
    ).ap()
    xC = nc.dram_tensor(
        "xC", [8, b_core // 2], dt.bfloat16, kind="ExternalInput"
    ).ap()
    w1a = nc.dram_tensor("w1a", [K1A, H], dt.bfloat16, kind="ExternalInput").ap()
    w1b = nc.dram_tensor("w1b", [128, H], dt.bfloat16, kind="ExternalInput").ap()
    w1c = nc.dram_tensor("w1c", [128, H], dt.bfloat16, kind="ExternalInput").ap()
    w2t = nc.dram_tensor("w2t", [H, H], dt.bfloat16, kind="ExternalInput").ap()
    w3t = nc.dram_tensor("w3t", [H, H], dt.bfloat16, kind="ExternalInput").ap()
    w4a = nc.dram_tensor("w4a", [H, 128], dt.bfloat16, kind="ExternalInput").ap()
    w4b = nc.dram_tensor("w4b", [H, 128], dt.bfloat16, kind="ExternalInput").ap()
    b1d = nc.dram_tensor("b1", [H, 1], dt.float32, kind="ExternalInput").ap()
    b2d = nc.dram_tensor("b2", [H, 1], dt.float32, kind="ExternalInput").ap()
    b3d = nc.dram_tensor("b3", [H, 1], dt.float32, kind="ExternalInput").ap()
    # strip-layout output: row 32u+10j+p, cols pk*TB+c  <->  y[(pk*8+4j+u)*TB+c, p]
    yTS = nc.dram_tensor(
        "yTS", [H, n_packs * TB], dt.bfloat16, kind="ExternalOutput"
    ).ap()

    relu = mybir.ActivationFunctionType.Relu

    # chunked loads; first chunks smaller for a faster pipeline start
    xa_chunks = [2048, 2048] + [4096] * ((b_core - 4096) // 4096)
    xb_chunks = [2048, 2048] + [4096] * ((b_core // 2 - 4096) // 4096)

    with tile.TileContext(nc) as tc:
        with (
            tc.tile_pool(name="wpool", bufs=1) as wpool,
            tc.tile_pool(name="xa", bufs=3) as xa_pool,
            tc.tile_pool(name="xb", bufs=3) as xb_pool,
            tc.tile_pool(name="h1p", bufs=9) as h1_pool,
            tc.tile_pool(name="h2p", bufs=9) as h2_pool,
            tc.tile_pool(name="h3p", bufs=14) as h3_pool,
            tc.tile_pool(name="yo", bufs=3) as y_pool,
            tc.tile_pool(name="ps1", bufs=3, space="PSUM") as ps1,
            tc.tile_pool(name="ps2", bufs=2, space="PSUM") as ps2,
            tc.tile_pool(name="ps3", bufs=2, space="PSUM") as ps3,
            tc.tile_pool(name="ps4", bufs=1, space="PSUM") as ps4,
        ):
            # --- resident loads: weights, biases, xC ---
            w1a_sb = wpool.tile([K1A, H], dt.bfloat16)
            nc.gpsimd.dma_start(w1a_sb[:], w1a[:, :])
            w1b_sb = wpool.tile([128, H], dt.bfloat16)
            nc.gpsimd.dma_start(w1b_sb[:], w1b[:, :])
            w1c_sb = wpool.tile([128, H], dt.bfloat16)
            nc.gpsimd.dma_start(w1c_sb[:], w1c[:, :])
            w2_sb = wpool.tile([H, H], dt.bfloat16)
            nc.gpsimd.dma_start(w2_sb[:], w2t[:, :])
            w3_sb = wpool.tile([H, H], dt.bfloat16)
            nc.gpsimd.dma_start(w3_sb[:], w3t[:, :])
            w4_sb = [
                wpool.tile([H, 128], dt.bfloat16, tag=f"w4_{j}", name=f"w4_{j}")
                for j in range(2)
            ]
            nc.gpsimd.dma_start(w4_sb[0][:], w4a[:, :])
            nc.gpsimd.dma_start(w4_sb[1][:], w4b[:, :])
            b_sb = []
            for j, bd in enumerate((b1d, b2d, b3d)):
                b = wpool.tile([H, 1], dt.float32, tag=f"b_{j}", name=f"b_{j}")
                nc.gpsimd.dma_start(b[:], bd[:, :])
                b_sb.append(b)
            # xC resident: partitions {0..3} even tile dims 192..195,
            # {32..35} odd tile; cols = pair*TB + c. Loaded in chunks
            # alongside xB so the first pairs' data lands fast.
            xc_sb = wpool.tile([128, b_core // 2], dt.bfloat16)

            evac_ctr = [0]

            def relu_evac(h_out, psum_in, bias_sb):
                use_act = evac_ctr[0] % 2 == 0
                evac_ctr[0] += 1
                if use_act:
                    return nc.scalar.activation(
                        h_out[:], psum_in, relu, bias=bias_sb[:]
                    )
                else:
                    return nc.vector.tensor_scalar(
                        h_out[:],
                        psum_in,
                        bias_sb[:],
                        0.0,
                        mybir.AluOpType.add,
                        mybir.AluOpType.max,
                    )

            # --- load bookkeeping ---
            xa_t: dict = {}   # tile -> (sbuf tile, col offset)
            xb_t: dict = {}   # pair -> (sbuf tile, col offset)
            xa_pos = [0, 0]
            xb_pos = [0, 0]

            def emit_xa_load():
                if xa_pos[0] >= len(xa_chunks):
                    return
                w = xa_chunks[xa_pos[0]]
                c0 = xa_pos[1]
                xa = xa_pool.tile(
                    [128, w], dt.bfloat16, tag="xa", name=f"xa_{xa_pos[0]}",
                    padded_shape=[128, 4096],
                )
                nc.sync.dma_start(xa[:], xA[:, c0 : c0 + w])
                for t in range(c0 // TB, (c0 + w) // TB):
                    xa_t[t] = (xa, t * TB - c0)
                xa_pos[0] += 1
                xa_pos[1] += w

            def emit_xb_load():
                if xb_pos[0] >= len(xb_chunks):
                    return
                w = xb_chunks[xb_pos[0]]
                c0 = xb_pos[1]
                xb = xb_pool.tile(
                    [128, w], dt.bfloat16, tag="xb", name=f"xb_{xb_pos[0]}",
                    padded_shape=[128, 4096],
                )
                nc.sync.dma_start(xb[:], xB[:, c0 : c0 + w])
                nc.sync.dma_start(xc_sb[0:4, c0 : c0 + w], xC[0:4, c0 : c0 + w])
                nc.sync.dma_start(xc_sb[32:36, c0 : c0 + w], xC[4:8, c0 : c0 + w])
                for p in range(c0 // TB, (c0 + w) // TB):
                    xb_t[p] = (xb, p * TB - c0)
                xb_pos[0] += 1
                xb_pos[1] += w

            h1_t: dict = {}
            h2_t: dict = {}
            h3_t: dict = {}

            def stage_A(i):
                # L1 for pair i: A (K=128) per tile, then the pair's B
                # (row-tiled K=64 x2) and C (row-tiled K=4 x2) passes.
                xb, boff = xb_t.pop(i)
                ps = []
                for q in range(2):
                    t = 2 * i + q
                    xa, off = xa_t[t]
                    p1 = ps1.tile([H, TB], dt.float32, tag="p1", name=f"p1_{t}")
                    nc.tensor.matmul(
                        p1[:], w1a_sb[:], xa[:, off : off + TB],
                        start=True, stop=False,
                    )
                    ps.append(p1)
                for q in range(2):
                    nc.tensor.matmul(
                        ps[q][:],
                        w1b_sb[64 * q : 64 * q + 64, :],
                        xb[64 * q : 64 * q + 64, boff : boff + TB],
                        start=False, stop=False,
                        tile_position=(64 * q, 0),
                        skip_group_check=True,
                    )
                for q in range(2):
                    nc.tensor.matmul(
                        ps[q][:],
                        w1c_sb[32 * q : 32 * q + 4, :],
                        xc_sb[32 * q : 32 * q + 4, i * TB : (i + 1) * TB],
                        start=False, stop=True,
                        tile_position=(32 * q, 0),
                        skip_group_check=True,
                    )
                for q in range(2):
                    t = 2 * i + q
                    h1 = h1_pool.tile([H, TB], dt.bfloat16, tag="h1", name=f"h1_{t}")
                    relu_evac(h1, ps[q][:], b_sb[0])
                    h1_t[t] = h1

            def stage_B(i):  # L2 for pair i
                for q in range(2):
                    t = 2 * i + q
                    h1 = h1_t.pop(t)
                    p2 = ps2.tile([H, TB], dt.float32, tag="p2", name=f"p2_{t}")
                    nc.tensor.matmul(p2[:], w2_sb[:], h1[:], start=True, stop=True)
                    h2 = h2_pool.tile([H, TB], dt.bfloat16, tag="h2", name=f"h2_{t}")
                    relu_evac(h2, p2[:], b_sb[1])
                    h2_t[t] = h2

            def stage_C(i):  # L3 for pair i
                for q in range(2):
                    t = 2 * i + q
                    h2 = h2_t.pop(t)
                    p3 = ps3.tile([H, TB], dt.float32, tag="p3", name=f"p3_{t}")
                    nc.tensor.matmul(p3[:], w3_sb[:], h2[:], start=True, stop=True)
                    h3 = h3_pool.tile([H, TB], dt.bfloat16, tag="h3", name=f"h3_{t}")
                    e3 = relu_evac(h3, p3[:], b_sb[2])
                    h3_t[t] = (h3, e3)

            p4_t: dict = {}

            def stage_Hj(pk, j):
                # head burst: variant j covers tiles 8pk+4j+u (u=0..3) --
                # pairs 4pk+2j, 4pk+2j+1, both freshly evacuated, so the 4
                # col-tiled matmuls co-issue. Stationary = [128,128] image
                # holding 4 copies of the zero-masked W4 variant.
                if j == 0:
                    p4_t[pk] = ps4.tile([H, TB], dt.float32, tag="p4", name=f"p4_{pk}")
                p4 = p4_t[pk]
                tiles = [8 * pk + 4 * j + u for u in range(4)]
                hs = [h3_t[t] for t in tiles]
                ldw = nc.tensor.ldweights(w4_sb[j][:])
                for _, e3 in hs:
                    _add_dep_helper(ldw.ins, e3.ins, True, "head ldw after e3")
                for u in range(4):
                    mm = nc.tensor.matmul(
                        p4[32 * u : 32 * u + 32, :],
                        w4_sb[j][:, 32 * u : 32 * u + 32],
                        hs[u][0][:],
                        start=(j == 0),
                        stop=(j == 1),
                        tile_position=(0, 32 * u),
                        skip_group_check=True,
                    )
                    mm.ins.ldweights = False
                    _add_dep_helper(mm.ins, ldw.ins, False, "head mm after ldw")
                for t in tiles:
                    h3_t.pop(t)
                if j == 1:
                    p4 = p4_t.pop(pk)
                    ysb = y_pool.tile([H, TB], dt.bfloat16, tag="ysb", name=f"ysb_{pk}")
                    nc.scalar.copy(ysb[:], p4[:])
                    nc.gpsimd.dma_start(yTS[:, pk * TB : (pk + 1) * TB], ysb[:])

            # --- software-pipelined emission (pair granularity, as in the
            # original: L1 of pair i, L2 of pair i-2, L3/head of pair i-4) ---
            emit_xa_load()   # tiles 0..3   (pairs 0-1)
            emit_xa_load()   # tiles 4..7   (pairs 2-3)
            emit_xb_load()   # pairs 0..3
            emit_xb_load()   # pairs 4..7
            for step in range(n_pairs + 4):
                # xa 4096-chunk = 8 tiles = 4 pairs -> one load per 4 steps;
                # xb 4096-chunk = 8 pairs -> one load per 8 steps.
                if step % 4 == 0:
                    emit_xa_load()
                if step % 8 == 4:
                    emit_xb_load()
                ib = step - 2
                if 0 <= ib < n_pairs:
                    stage_B(ib)
                ic = step - 4
                if 0 <= ic < n_pairs:
                    stage_C(ic)
                    if ic % 4 == 1:
                        stage_Hj(ic // 4, 0)
                    elif ic % 4 == 3:
                        stage_Hj(ic // 4, 1)
                if step < n_pairs:
                    stage_A(step)

    nc.compile()
    return nc


def _prep_core_inputs(x_shard: np.ndarray, weights: dict) -> dict:
    b = x_shard.shape[0]
    xb16 = x_shard.astype(BF16)
    xA = np.ascontiguousarray(xb16[:, 0:128].T)
    v = xb16[:, 128:192].reshape(b // 1024, 2, TB, 64)
    xB = np.ascontiguousarray(v.transpose(1, 3, 0, 2).reshape(128, b // 2))
    w = xb16[:, 192:196].reshape(b // 1024, 2, TB, 4)
    xC = np.ascontiguousarray(w.transpose(1, 3, 0, 2).reshape(8, b // 2))
    return {"xA": xA, "xB": xB, "xC": xC, **weights}


def _prep_weights(W1, b1, W2, b2, W3, b3, W4) -> dict:
    s1 = np.sign(W1).astype(np.float32)
    w1b = np.zeros((128, H), np.float32)
    w1b[0:64] = s1[:, 128:192].T
    w1b[64:128] = s1[:, 128:192].T
    w1c = np.zeros((128, H), np.float32)
    w1c[0:4] = s1[:, 192:196].T
    w1c[32:36] = s1[:, 192:196].T
    w4a = np.zeros((32, H), np.float32)
    w4a[0:D_OUT] = W4
    w4b = np.zeros((32, H), np.float32)
    w4b[D_OUT : 2 * D_OUT] = W4
    return {
        "w1a": np.ascontiguousarray(s1[:, 0:128].T).astype(BF16),
        "w1b": w1b.astype(BF16),
        "w1c": w1c.astype(BF16),
        "w2t": np.ascontiguousarray(np.sign(W2).T).astype(BF16),
        "w3t": np.ascontiguousarray(np.sign(W3).T).astype(BF16),
        "w4a": np.ascontiguousarray(np.tile(w4a.T, (1, 4))).astype(BF16),
        "w4b": np.ascontiguousarray(np.tile(w4b.T, (1, 4))).astype(BF16),
        "b1": b1.reshape(H, 1).astype(np.float32),
        "b2": b2.reshape(H, 1).astype(np.float32),
        "b3": b3.reshape(H, 1).astype(np.float32),
    }


def _unscramble(yTS: np.ndarray, b_core: int) -> np.ndarray:
    """yTS [128, n_packs*TB] strip layout -> y_core [b_core, 10]."""
    n_packs = b_core // (PACK * TB)
    yf = np.asarray(yTS, dtype=np.float32)
    # yTS[32u+10j+p, pk*TB+c] = y[(pk*8+4j+u)*TB + c, p]
    v = yf.reshape(4, 32, n_packs, TB)[:, :20]  # [u, 10j+p, pk, c]
    v = v.reshape(4, 2, 10, n_packs, TB)  # [u, j, p, pk, c]
    y = v.transpose(3, 1, 0, 4, 2).reshape(b_core, D_OUT)
    return y


_NC_CACHE: dict = {}


def run(x, W1, b1, W2, b2, W3, b3, W4, b4, trace=False, trace_kwargs=None):
    """Run the SPMD kernel on 8 cores; returns (y, BassKernelResults)."""
    x = np.asarray(x, dtype=np.float32)
    b_total = x.shape[0]
    assert b_total % N_CORES == 0
    b_core = b_total // N_CORES

    key = b_core
    if key not in _NC_CACHE:
        _NC_CACHE[key] = build_nc(b_core)
    nc = _NC_CACHE[key]

    weights = _prep_weights(
        np.asarray(W1), np.asarray(b1), np.asarray(W2), np.asarray(b2),
        np.asarray(W3), np.asarray(b3), np.asarray(W4),
    )
    in_maps = [
        _prep_core_inputs(x[c * b_core : (c + 1) * b_core], weights)
        for c in range(N_CORES)
    ]
    res = run_bass_kernel_spmd(
        nc,
        in_maps,
        list(range(N_CORES)),
        trace=trace,
        **(trace_kwargs or {}),
    )
    b4f = np.asarray(b4, dtype=np.float32)
    y = np.empty((b_total, D_OUT), dtype=np.float32)
    for c in range(N_CORES):
        y[c * b_core : (c + 1) * b_core] = _unscramble(res.results[c]["yTS"], b_core)
    y += b4f
    return y, res


def kernel(x, W1, b1, W2, b2, W3, b3, W4, b4):
    y, _ = run(x, W1, b1, W2, b2, W3, b3, W4, b4)
    return y


# revision 11
# speedup vs baseline: 1.5484x; 1.0113x over previous
"""Trainium2 Bass kernel for BinaryMLP.

reference:
    h = relu(x @ sign(W1).T + b1)   # [B, 128], x: [B, 196]
    h = relu(h @ sign(W2).T + b2)   # [B, 128]
    h = relu(h @ sign(W3).T + b3)   # [B, 128]
    y = h @ W4.T + b4               # [B, 10] (full-precision head)

Strategy (pure data parallel over 8 cores, 65536 rows each):
  - Host packs the x shard into three bf16 DRAM tensors so every large
    DMA spans all 128 SBUF partitions (a [68, N] transfer lands on only
    4 of 16 SDMA engines and was the original bandwidth ceiling):
      xA [128, B]    x dims 0..127, batch-major columns
      xB [128, B/2]  x dims 128..191, tile-pair packed (even tile of the
                     pair on partitions 0..63, odd tile on 64..127)
      xC [8, B/2]    x dims 192..195, pair packed on partition groups
                     {0..3} (even tile) and {32..35} (odd tile); kept
                     resident in SBUF for the whole kernel
  - L1 contraction split K=196 = 128 + 64 + 4 per tile:
      pass A: full-array K=128 matmul
      pass B: the pair's two K=64 remainders run concurrently via row
              tiling at tile_position (0,0) / (64,0); the stationary
              operand is a single [128,128] image holding W1b twice
      pass C: the pair's two K=4 leftovers run concurrently at
              (0,0) / (32,0) from a [128,128] image holding W1c copies
    The framework emits a correctly-positioned LDWEIGHTS before every
    matmul; the PE's 64-deep reorder window hides them.
  - PSUM: ps1 bufs=3 (L1), ps2/ps3 bufs=2 (L2/L3), 1 head bank.
    All relu+bias evacuations are [128,512], alternating ScalarE and
    VectorE (fp32 PSUM reads run at 1 elem/lane/cycle on both engines,
    which makes the two evacuation engines the throughput wall).
  - Software pipelining: per-pair stages with a 2-pair skew (L2 of pair
    i-2, L1 of pair i, L3/head of pair i-4 per step).
  - Head (M=10): 8 tiles per PSUM bank via 4x column tiling x 2
    zero-masked W4 variants packed in a [128,128] stationary image.
    Output stays in the strip layout yTS[128, .] (rows 32u+10j+p),
    stored bf16; the host unscrambles and adds b4.
"""

import numpy as np
import ml_dtypes

import concourse.bass as bass
from concourse.bass import _add_dep_helper
import concourse.mybir as mybir
import concourse.tile as tile
from concourse import bacc
from concourse.bass_utils import run_bass_kernel_spmd

BF16 = ml_dtypes.bfloat16

B_FULL, D_IN, H, D_OUT = 524288, 196, 128, 10
N_CORES = 8
TB = 512          # batch tile = matmul free dim (one PSUM bank of fp32)
PACK = 8          # tiles per head pack / store group
K1A = 128


def build_nc(b_core: int, n_cores: int = N_CORES):
    """Build the per-core Bass program (SPMD: same program on all cores)."""
    dt = mybir.dt
    nc = bacc.Bacc(
        "TRN2", target_bir_lowering=False, debug=False, num_devices=n_cores
    )

    n_tiles = b_core // TB
    assert b_core % (PACK * TB) == 0
    n_packs = n_tiles // PACK
    n_pairs = n_tiles // 2

    xA = nc.dram_tensor("xA", [128, b_core], dt.bfloat16, kind="ExternalInput").ap()
    xB = nc.dram_tensor(
        "xB", [128, b_core // 2], dt.bfloat16, kind="ExternalInput"
    ).ap()
    xC = nc.dram_tensor(
        "xC", [8, b_core // 2], dt.bfloat16, kind="ExternalInput"
    ).ap()
    w1a = nc.dram_tensor("w1a", [K1A, H], dt.bfloat16, kind="ExternalInput").ap()
    w1b = nc.dram_tensor("w1b", [128, H], dt.bfloat16, kind="ExternalInput").ap()
    w1c = nc.dram_tensor("w1c", [128, H], dt.bfloat16, kind="ExternalInput").ap()
    w2t = nc.dram_tensor("w2t", [H, H], dt.bfloat16, kind="ExternalInput").ap()
    w3t = nc.dram_tensor("w3t", [H, H], dt.bfloat16, kind="ExternalInput").ap()
    w4a = nc.dram_tensor("w4a", [H, 128], dt.bfloat16, kind="ExternalInput").ap()
    w4b = nc.dram_tensor("w4b", [H, 128], dt.bfloat16, kind="ExternalInput").ap()
    b1d = nc.dram_tensor("b1", [H, 1], dt.float32, kind="ExternalInput").ap()
    b2d = nc.dram_tensor("b2", [H, 1], dt.float32, kind="ExternalInput").ap()
    b3d = nc.dram_tensor("b3", [H, 1], dt.float32, kind="ExternalInput").ap()
    # strip-layout output: row 32u+10j+p, cols pk*TB+c  <->  y[(pk*8+4j+u)*TB+c, p]
    yTS = nc.dram_tensor(
        "yTS", [H, n_packs * TB], dt.bfloat16, kind="ExternalOutput"
    ).ap()

    relu = mybir.ActivationFunctionType.Relu

    # chunked loads; first chunks smaller for a faster pipeline start
    xa_chunks = [1024, 1024, 2048] + [4096] * ((b_core - 4096) // 4096)
    xb_chunks = [2048, 2048] + [4096] * ((b_core // 2 - 4096) // 4096)

    with tile.TileContext(nc) as tc:
        with (
            tc.tile_pool(name="wpool", bufs=1) as wpool,
            tc.tile_pool(name="xa", bufs=3) as xa_pool,
            tc.tile_pool(name="xb", bufs=3) as xb_pool,
            tc.tile_pool(name="h1p", bufs=9) as h1_pool,
            tc.tile_pool(name="h2p", bufs=9) as h2_pool,
            tc.tile_pool(name="h3p", bufs=14) as h3_pool,
            tc.tile_pool(name="yo", bufs=3) as y_pool,
            tc.tile_pool(name="ps1", bufs=3, space="PSUM") as ps1,
            tc.tile_pool(name="ps2", bufs=2, space="PSUM") as ps2,
            tc.tile_pool(name="ps3", bufs=2, space="PSUM") as ps3,
            tc.tile_pool(name="ps4", bufs=1, space="PSUM") as ps4,
        ):
            # --- resident loads: weights, biases, xC ---
            w1a_sb = wpool.tile([K1A, H], dt.bfloat16)
            nc.gpsimd.dma_start(w1a_sb[:], w1a[:, :])
            w1b_sb = wpool.tile([128, H], dt.bfloat16)
            nc.gpsimd.dma_start(w1b_sb[:], w1b[:, :])
            w1c_sb = wpool.tile([128, H], dt.bfloat16)
            nc.gpsimd.dma_start(w1c_sb[:], w1c[:, :])
            w2_sb = wpool.tile([H, H], dt.bfloat16)
            nc.gpsimd.dma_start(w2_sb[:], w2t[:, :])
            w3_sb = wpool.tile([H, H], dt.bfloat16)
            nc.gpsimd.dma_start(w3_sb[:], w3t[:, :])
            w4_sb = [
                wpool.tile([H, 128], dt.bfloat16, tag=f"w4_{j}", name=f"w4_{j}")
                for j in range(2)
            ]
            nc.gpsimd.dma_start(w4_sb[0][:], w4a[:, :])
            nc.gpsimd.dma_start(w4_sb[1][:], w4b[:, :])
            b_sb = []
            for j, bd in enumerate((b1d, b2d, b3d)):
                b = wpool.tile([H, 1], dt.float32, tag=f"b_{j}", name=f"b_{j}")
                nc.gpsimd.dma_start(b[:], bd[:, :])
                b_sb.append(b)
            # xC resident: partitions {0..3} even tile dims 192..195,
            # {32..35} odd tile; cols = pair*TB + c. Loaded in chunks
            # alongside xB so the first pairs' data lands fast.
            xc_sb = wpool.tile([128, b_core // 2], dt.bfloat16)

            # HAM warm-up: dependency-free matmuls on a zeroed SBUF tile
            # keep the PE busy through the initial DMA wait so the clock
            # gate reaches K=8/8 before real data arrives.
            wz = wpool.tile([128, TB], dt.bfloat16)
            nc.vector.memset(wz[:], 0.0)
            pz = ps4.tile([H, TB], dt.float32, tag="p4", name="p4_warm")
            for _wk in range(24):
                nc.tensor.matmul(pz[:], wz[:, 0:128], wz[:], start=True, stop=True)

            evac_ctr = [0]

            def relu_evac(h_out, psum_in, bias_sb):
                use_act = evac_ctr[0] % 2 == 0
                evac_ctr[0] += 1
                if use_act:
                    return nc.scalar.activation(
                        h_out[:], psum_in, relu, bias=bias_sb[:]
                    )
                else:
                    return nc.vector.tensor_scalar(
                        h_out[:],
                        psum_in,
                        bias_sb[:],
                        0.0,
                        mybir.AluOpType.add,
                        mybir.AluOpType.max,
                    )

            # --- load bookkeeping ---
            xa_t: dict = {}   # tile -> (sbuf tile, col offset)
            xb_t: dict = {}   # pair -> (sbuf tile, col offset)
            xa_pos = [0, 0]
            xb_pos = [0, 0]

            def emit_xa_load():
                if xa_pos[0] >= len(xa_chunks):
                    return
                w = xa_chunks[xa_pos[0]]
                c0 = xa_pos[1]
                xa = xa_pool.tile(
                    [128, w], dt.bfloat16, tag="xa", name=f"xa_{xa_pos[0]}",
                    padded_shape=[128, 4096],
                )
                nc.sync.dma_start(xa[:], xA[:, c0 : c0 + w])
                for t in range(c0 // TB, (c0 + w) // TB):
                    xa_t[t] = (xa, t * TB - c0)
                xa_pos[0] += 1
                xa_pos[1] += w

            def emit_xb_load():
                if xb_pos[0] >= len(xb_chunks):
                    return
                w = xb_chunks[xb_pos[0]]
                c0 = xb_pos[1]
                xb = xb_pool.tile(
                    [128, w], dt.bfloat16, tag="xb", name=f"xb_{xb_pos[0]}",
                    padded_shape=[128, 4096],
                )
                nc.sync.dma_start(xb[:], xB[:, c0 : c0 + w])
                nc.sync.dma_start(xc_sb[0:4, c0 : c0 + w], xC[0:4, c0 : c0 + w])
                nc.sync.dma_start(xc_sb[32:36, c0 : c0 + w], xC[4:8, c0 : c0 + w])
                for p in range(c0 // TB, (c0 + w) // TB):
                    xb_t[p] = (xb, p * TB - c0)
                xb_pos[0] += 1
                xb_pos[1] += w

            h1_t: dict = {}
            h2_t: dict = {}
            h3_t: dict = {}

            def stage_A(i):
                # L1 for pair i: A (K=128) per tile, then the pair's B
                # (row-tiled K=64 x2) and C (row-tiled K=4 x2) passes.
                xb, boff = xb_t.pop(i)
                ps = []
                for q in range(2):
                    t = 2 * i + q
                    xa, off = xa_t[t]
                    p1 = ps1.tile([H, TB], dt.float32, tag="p1", name=f"p1_{t}")
                    nc.tensor.matmul(
                        p1[:], w1a_sb[:], xa[:, off : off + TB],
                        start=True, stop=False,
                    )
                    ps.append(p1)
                for q in range(2):
                    nc.tensor.matmul(
                        ps[q][:],
                        w1b_sb[64 * q : 64 * q + 64, :],
                        xb[64 * q : 64 * q + 64, boff : boff + TB],
                        start=False, stop=False,
                        tile_position=(64 * q, 0),
                        skip_group_check=True,
                    )
                for q in range(2):
                    nc.tensor.matmul(
                        ps[q][:],
                        w1c_sb[32 * q : 32 * q + 4, :],
                        xc_sb[32 * q : 32 * q + 4, i * TB : (i + 1) * TB],
                        start=False, stop=True,
                        tile_position=(32 * q, 0),
                        skip_group_check=True,
                    )
                for q in range(2):
                    t = 2 * i + q
                    h1 = h1_pool.tile([H, TB], dt.bfloat16, tag="h1", name=f"h1_{t}")
                    relu_evac(h1, ps[q][:], b_sb[0])
                    h1_t[t] = h1

            def stage_B(i):  # L2 for pair i
                for q in range(2):
                    t = 2 * i + q
                    h1 = h1_t.pop(t)
                    p2 = ps2.tile([H, TB], dt.float32, tag="p2", name=f"p2_{t}")
                    nc.tensor.matmul(p2[:], w2_sb[:], h1[:], start=True, stop=True)
                    h2 = h2_pool.tile([H, TB], dt.bfloat16, tag="h2", name=f"h2_{t}")
                    relu_evac(h2, p2[:], b_sb[1])
                    h2_t[t] = h2

            def stage_C(i):  # L3 for pair i
                for q in range(2):
                    t = 2 * i + q
                    h2 = h2_t.pop(t)
                    p3 = ps3.tile([H, TB], dt.float32, tag="p3", name=f"p3_{t}")
                    nc.tensor.matmul(p3[:], w3_sb[:], h2[:], start=True, stop=True)
                    h3 = h3_pool.tile([H, TB], dt.bfloat16, tag="h3", name=f"h3_{t}")
                    e3 = relu_evac(h3, p3[:], b_sb[2])
                    h3_t[t] = (h3, e3)

            p4_t: dict = {}

            def stage_Hj(pk, j):
                # head burst: variant j covers tiles 8pk+4j+u (u=0..3) --
                # pairs 4pk+2j, 4pk+2j+1, both freshly evacuated, so the 4
                # col-tiled matmuls co-issue. Stationary = [128,128] image
                # holding 4 copies of the zero-masked W4 variant.
                if j == 0:
                    p4_t[pk] = ps4.tile([H, TB], dt.float32, tag="p4", name=f"p4_{pk}")
                p4 = p4_t[pk]
                tiles = [8 * pk + 4 * j + u for u in range(4)]
                hs = [h3_t[t] for t in tiles]
                ldw = nc.tensor.ldweights(w4_sb[j][:])
                for _, e3 in hs:
                    _add_dep_helper(ldw.ins, e3.ins, True, "head ldw after e3")
                for u in range(4):
                    mm = nc.tensor.matmul(
                        p4[32 * u : 32 * u + 32, :],
                        w4_sb[j][:, 32 * u : 32 * u + 32],
                        hs[u][0][:],
                        start=(j == 0),
                        stop=(j == 1),
                        tile_position=(0, 32 * u),
                        skip_group_check=True,
                    )
                    mm.ins.ldweights = False
                    _add_dep_helper(mm.ins, ldw.ins, False, "head mm after ldw")
                for t in tiles:
                    h3_t.pop(t)
                if j == 1:
                    p4 = p4_t.pop(pk)
                    ysb = y_pool.tile([H, TB], dt.bfloat16, tag="ysb", name=f"ysb_{pk}")
                    nc.scalar.copy(ysb[:], p4[:])
                    nc.gpsimd.dma_start(yTS[:, pk * TB : (pk + 1) * TB], ysb[:])

            # --- software-pipelined emission (pair granularity, as in the
            # original: L1 of pair i, L2 of pair i-2, L3/head of pair i-4) ---
            emit_xa_load()   # tiles 0..3   (pairs 0-1)
            emit_xa_load()   # tiles 4..7   (pairs 2-3)
            emit_xb_load()   # pairs 0..3
            emit_xb_load()   # pairs 4..7
            for step in range(n_pairs + 4):
                # xa 4096-chunk = 8 tiles = 4 pairs -> one load per 4 steps;
                # xb 4096-chunk = 8 pairs -> one load per 8 steps.
                if step in (0, 1) or step % 4 == 2:
                    emit_xa_load()
                if step % 8 == 4:
                    emit_xb_load()
                ib = step - 2
                if 0 <= ib < n_pairs:
                    stage_B(ib)
                ic = step - 4
                if 0 <= ic < n_pairs:
                    stage_C(ic)
                    if ic % 4 == 1:
                        stage_Hj(ic // 4, 0)
                    elif ic % 4 == 3:
                        stage_Hj(ic // 4, 1)
                if step < n_pairs:
                    stage_A(step)

    nc.compile()
    return nc


def _prep_core_inputs(x_shard: np.ndarray, weights: dict) -> dict:
    b = x_shard.shape[0]
    xb16 = x_shard.astype(BF16)
    xA = np.ascontiguousarray(xb16[:, 0:128].T)
    v = xb16[:, 128:192].reshape(b // 1024, 2, TB, 64)
    xB = np.ascontiguousarray(v.transpose(1, 3, 0, 2).reshape(128, b // 2))
    w = xb16[:, 192:196].reshape(b // 1024, 2, TB, 4)
    xC = np.ascontiguousarray(w.transpose(1, 3, 0, 2).reshape(8, b // 2))
    return {"xA": xA, "xB": xB, "xC": xC, **weights}


def _prep_weights(W1, b1, W2, b2, W3, b3, W4) -> dict:
    s1 = np.sign(W1).astype(np.float32)
    w1b = np.zeros((128, H), np.float32)
    w1b[0:64] = s1[:, 128:192].T
    w1b[64:128] = s1[:, 128:192].T
    w1c = np.zeros((128, H), np.float32)
    w1c[0:4] = s1[:, 192:196].T
    w1c[32:36] = s1[:, 192:196].T
    w4a = np.zeros((32, H), np.float32)
    w4a[0:D_OUT] = W4
    w4b = np.zeros((32, H), np.float32)
    w4b[D_OUT : 2 * D_OUT] = W4
    return {
        "w1a": np.ascontiguousarray(s1[:, 0:128].T).astype(BF16),
        "w1b": w1b.astype(BF16),
        "w1c": w1c.astype(BF16),
        "w2t": np.ascontiguousarray(np.sign(W2).T).astype(BF16),
        "w3t": np.ascontiguousarray(np.sign(W3).T).astype(BF16),
        "w4a": np.ascontiguousarray(np.tile(w4a.T, (1, 4))).astype(BF16),
        "w4b": np.ascontiguousarray(np.tile(w4b.T, (1, 4))).astype(BF16),
        "b1": b1.reshape(H, 1).astype(np.float32),
        "b2": b2.reshape(H, 1).astype(np.float32),
        "b3": b3.reshape(H, 1).astype(np.float32),
    }


def _unscramble(yTS: np.ndarray, b_core: int) -> np.ndarray:
    """yTS [128, n_packs*TB] strip layout -> y_core [b_core, 10]."""
    n_packs = b_core // (PACK * TB)
    yf = np.asarray(yTS, dtype=np.float32)
    # yTS[32u+10j+p, pk*TB+c] = y[(pk*8+4j+u)*TB + c, p]
    v = yf.reshape(4, 32, n_packs, TB)[:, :20]  # [u, 10j+p, pk, c]
    v = v.reshape(4, 2, 10, n_packs, TB)  # [u, j, p, pk, c]
    y = v.transpose(3, 1, 0, 4, 2).reshape(b_core, D_OUT)
    return y


_NC_CACHE: dict = {}


def run(x, W1, b1, W2, b2, W3, b3, W4, b4, trace=False, trace_kwargs=None):
    """Run the SPMD kernel on 8 cores; returns (y, BassKernelResults)."""
    x = np.asarray(x, dtype=np.float32)
    b_total = x.shape[0]
    assert b_total % N_CORES == 0
    b_core = b_total // N_CORES

    key = b_core
    if key not in _NC_CACHE:
        _NC_CACHE[key] = build_nc(b_core)
    nc = _NC_CACHE[key]

    weights = _prep_weights(
        np.asarray(W1), np.asarray(b1), np.asarray(W2), np.asarray(b2),
        np.asarray(W3), np.asarray(b3), np.asarray(W4),
    )
    in_maps = [
        _prep_core_inputs(x[c * b_core : (c + 1) * b_core], weights)
        for c in range(N_CORES)
    ]
    res = run_bass_kernel_spmd(
        nc,
        in_maps,
        list(range(N_CORES)),
        trace=trace,
        **(trace_kwargs or {}),
    )
    b4f = np.asarray(b4, dtype=np.float32)
    y = np.empty((b_total, D_OUT), dtype=np.float32)
    for c in range(N_CORES):
        y[c * b_core : (c + 1) * b_core] = _unscramble(res.results[c]["yTS"], b_core)
    y += b4f
    return y, res


def kernel(x, W1, b1, W2, b2, W3, b3, W4, b4):
    y, _ = run(x, W1, b1, W2, b2, W3, b3, W4, b4)
    return y
